# revision 25
# baseline (speedup 1.0000x reference)
"""Trainium2 Bass kernel for AttentionConditionGenerator.

Reference computation (per row b of B=16384):
    kv = [h_u_cross[b], h_u_target[b]]            # (2, 1024)
    q  = dom_movie @ w_q.T + b_q                  # fixed across rows
    scores = (q/8) . k[s],  attn = softmax_2(scores)
    ctx = attn0*v0 + attn1*v1 ; y = ctx @ w_o.T + b_o
    x = LN1(dom_movie + y); h = gelu(x @ w1.T + b1)
    out = LN2(x + h @ w2.T + b2)

Algebraic folding used here (exact, done on host in fp64/fp32):
  - q is row-independent -> scores fold to A @ kv with A[h,:] = sum_{j in head h} qs[j] w_k[j,:]
    (b_k cancels in the 2-way softmax difference); attn0 = sigmoid(A @ (xc - xt)).
  - ctx = v_t + attn0 * v_d with v_t = w_v@xt + b_v, v_d = w_v@(xc - xt)
    (b_v cancels in v_d); the constant b_v path is folded into the output
    bias: bod2 = b_o + dom_movie + w_o @ b_v.

Device mapping: batch split over 8 cores (2048 rows each). Activations flow
feature-major (features on partitions, rows on the free axis) through all
matmuls (bf16 operands, fp32 PSUM accumulation); LayerNorms run row-major
using bn_stats after a TensorE transpose. Weights are pre-transposed and
pre-tiled on the host; weight matrices are streamed per 512-row block in
half-width slices to fit SBUF.
"""

import numpy as np
import ml_dtypes

try:
    import concourse.bass as bass
except ImportError:  # pragma: no cover - path setup for fresh environments
    import sys

    for _p in ("/opt/trn_rl_repo", "/root/.axon_site/_ro/trn_rl_repo"):
        if _p not in sys.path:
            sys.path.insert(0, _p)
    import concourse.bass as bass

import concourse.mybir as mybir
import concourse.tile as tile
from concourse import bacc
from concourse.bass_utils import run_bass_kernel_spmd
from concourse.masks import make_identity

F32 = mybir.dt.float32
BF16 = mybir.dt.bfloat16
NPBF16 = ml_dtypes.bfloat16

D = 1024
H = 16
HD = 64
FFN = 4096
EPS = 1e-5
N_CORES = 8
B_TOTAL = 16384
B_CORE = B_TOTAL // N_CORES  # 2048

KT = D // 128  # 8 feature k-tiles
MT = D // 128  # 8 output m-tiles
FMT = FFN // 128  # 32 FFN m-tiles
NMG = 4  # host-side FFN1 m-group axis (1024 cols each)


def build_program(b_core, r_blk, trivial_ln1, trivial_ln2, gelu_func=None):
    """Build and compile the per-core Bass program."""
    if gelu_func is None:
        gelu_func = mybir.ActivationFunctionType.Gelu
    nb = b_core // r_blk  # row blocks
    ni = r_blk // 128  # 128-row subtiles per block
    N = r_blk  # matmul moving (free) dim

    nc = bacc.Bacc("TRN2", target_bir_lowering=False)

    # ---- DRAM I/O ------------------------------------------------------
    xtb_d = nc.dram_tensor("xtb", [b_core, D], BF16, kind="ExternalInput")
    db_d = nc.dram_tensor("db", [b_core, D], BF16, kind="ExternalInput")
    wv_d = nc.dram_tensor("wvT", [128, KT, D], BF16, kind="ExternalInput")
    wo_d = nc.dram_tensor("woT", [128, KT, D], BF16, kind="ExternalInput")
    w1_d = nc.dram_tensor("w1P", [128, NMG, KT, D], BF16, kind="ExternalInput")
    w2_d = nc.dram_tensor("w2P", [128, MT, FMT, 128], BF16, kind="ExternalInput")
    at_d = nc.dram_tensor("AT", [128, KT, H], BF16, kind="ExternalInput")
    e_d = nc.dram_tensor("E", [H, MT, 128], BF16, kind="ExternalInput")
    bod_d = nc.dram_tensor("bod2", [128, MT], F32, kind="ExternalInput")
    b1_d = nc.dram_tensor("b1p", [128, FMT], F32, kind="ExternalInput")
    b2_d = nc.dram_tensor("b2p", [128, MT], F32, kind="ExternalInput")
    if not trivial_ln1:
        g1_d = nc.dram_tensor("g1", [D], F32, kind="ExternalInput")
        c1_d = nc.dram_tensor("c1", [D], F32, kind="ExternalInput")
    if not trivial_ln2:
        g2_d = nc.dram_tensor("g2", [D], F32, kind="ExternalInput")
        c2_d = nc.dram_tensor("c2", [D], F32, kind="ExternalInput")
    out_d = nc.dram_tensor("out", [b_core, D], F32, kind="ExternalOutput")

    def bcast_ap(dram):
        # [D] dram vector -> [128, D] partition-broadcast access pattern
        return bass.AP(tensor=dram, offset=0, ap=[[0, 128], [1, D]])

    with tile.TileContext(nc) as tc:
        with (
            tc.tile_pool(name="consts", bufs=1) as consts,
            tc.tile_pool(name="wstream", bufs=2) as wstream,
            tc.tile_pool(name="fm", bufs=1) as fm,
            tc.tile_pool(name="work", bufs=2) as work,
            tc.tile_pool(name="rm", bufs=2) as rm,
            tc.tile_pool(name="stats", bufs=3) as st,
            tc.tile_pool(name="ps_mm", bufs=3, space="PSUM") as ps_mm,
            tc.tile_pool(name="ps_tr", bufs=3, space="PSUM") as ps_tr,
            tc.tile_pool(name="ps_sc", bufs=1, space="PSUM") as ps_sc,
            tc.tile_pool(name="ps_ab", bufs=1, space="PSUM") as ps_ab,
        ):
            # ---- constants ------------------------------------------
            ident = consts.tile([128, 128], F32)
            make_identity(nc, ident)
            at_s = consts.tile([128, KT, H], BF16)
            nc.sync.dma_start(at_s, at_d[:, :, :])
            e_s = consts.tile([H, MT, 128], BF16)
            nc.sync.dma_start(e_s, e_d[:, :, :])
            bod_s = consts.tile([128, MT], F32)
            nc.sync.dma_start(bod_s, bod_d[:, :])
            b1_s = consts.tile([128, FMT], F32)
            nc.sync.dma_start(b1_s, b1_d[:, :])
            b2_s = consts.tile([128, MT], F32)
            nc.sync.dma_start(b2_s, b2_d[:, :])
            eps_s = consts.tile([128, 1], F32)
            nc.vector.memset(eps_s, EPS)
            g1_s = c1_s = g2_s = c2_s = None
            if not trivial_ln1:
                g1_s = consts.tile([128, D], F32)
                nc.gpsimd.dma_start(g1_s, bcast_ap(g1_d))
                c1_s = consts.tile([128, D], F32)
                nc.gpsimd.dma_start(c1_s, bcast_ap(c1_d))
            if not trivial_ln2:
                g2_s = consts.tile([128, D], F32)
                nc.gpsimd.dma_start(g2_s, bcast_ap(g2_d))
                c2_s = consts.tile([128, D], F32)
                nc.gpsimd.dma_start(c2_s, bcast_ap(c2_d))

            def layernorm_rm(y_i, out_tile, g_s, c_s):
                """Row-major LayerNorm of y_i [128, D] f32 -> out_tile."""
                stt = st.tile([128, 2, 6], F32, tag="bnst")
                nc.vector.bn_stats(stt[:, 0, :], y_i[:, 0:512])
                nc.vector.bn_stats(stt[:, 1, :], y_i[:, 512:1024])
                mv = st.tile([128, 2], F32, tag="bnmv")
                nc.vector.bn_aggr(mv, stt)
                std = st.tile([128, 1], F32, tag="bnstd")
                nc.scalar.activation(std, mv[:, 1:2],
                                     mybir.ActivationFunctionType.Sqrt,
                                     bias=eps_s)
                rstd = st.tile([128, 1], F32, tag="bnrstd")
                nc.vector.reciprocal(rstd, std)
                if g_s is None:
                    nc.vector.tensor_scalar(
                        out_tile, y_i, mv[:, 0:1], rstd,
                        op0=mybir.AluOpType.subtract, op1=mybir.AluOpType.mult)
                else:
                    xn = st.tile([128, D], F32, tag="bnxn")
                    nc.vector.tensor_scalar(
                        xn, y_i, mv[:, 0:1], rstd,
                        op0=mybir.AluOpType.subtract, op1=mybir.AluOpType.mult)
                    nc.vector.tensor_mul(out_tile, xn, g_s)
                    nc.vector.tensor_add(out_tile, out_tile, c_s)

            def stage0(blk):
                """DMA-transpose-load block blk inputs to feature-major
                bf16 (xtT, dT). xtb/db are precast bf16 on the host."""
                r0 = blk * r_blk
                xtT = fm.tile([128, KT, N], BF16, tag="xtT", bufs=2)
                dT = fm.tile([128, KT, N], BF16, tag="dT", bufs=2)
                for src, dst in ((db_d, dT), (xtb_d, xtT)):
                    for j in range(KT):
                        nc.sync.dma_start(
                            dst[:, j, :],
                            src[r0:r0 + N, 128 * j:128 * (j + 1)],
                            transpose=True)
                return xtT, dT

            def prefetch_vo(mh_w):
                """Queue one wv/wo half-slice load (interleaved into the
                FFN1 weight stream so next block's attention weights are
                resident before the block boundary)."""
                mh, w_d = mh_w
                ws = wstream.tile([128, KT, 512], BF16, tag="wVO", bufs=4,
                                  name="wvo")
                nc.sync.dma_start(ws, w_d[:, :, 512 * mh:512 * (mh + 1)])
                return ws

            def stage1(dT):
                """Attention scores -> attn0 = sigmoid(A @ d)."""
                psc = ps_sc.tile([H, N], F32)
                for k in range(KT):
                    nc.tensor.matmul(psc, at_s[:, k, :], dT[:, k, :],
                                     start=(k == 0), stop=(k == KT - 1))
                attn0 = st.tile([H, N], BF16, tag="attn0", bufs=2)
                nc.scalar.activation(attn0, psc,
                                     mybir.ActivationFunctionType.Sigmoid)
                return attn0

            # ---- main loop over row blocks (inputs, attention weights and
            # scores software-pipelined one block ahead) ----
            # First block: order the DMA queue by first use
            # (dT -> wv -> xtT -> wo) to shorten the cold start.
            xtT0 = fm.tile([128, KT, N], BF16, tag="xtT", bufs=2)
            dT0 = fm.tile([128, KT, N], BF16, tag="dT", bufs=2)
            for j in range(KT):
                nc.sync.dma_start(dT0[:, j, :], db_d[0:N, 128 * j:128 * (j + 1)],
                                  transpose=True)
            nxt_vo = [prefetch_vo(x) for x in ((0, wv_d), (1, wv_d))]
            for j in range(KT):
                nc.sync.dma_start(xtT0[:, j, :],
                                  xtb_d[0:N, 128 * j:128 * (j + 1)],
                                  transpose=True)
            nxt_vo += [prefetch_vo(x) for x in ((0, wo_d), (1, wo_d))]
            nxt = (xtT0, dT0)
            nxt_at = stage1(dT0)
            for blk in range(nb):
                r0 = blk * r_blk
                xtT, dT = nxt
                wv_sl, wo_sl = nxt_vo[0:2], nxt_vo[2:4]
                attn0 = nxt_at
                nxt_vo = []

                # stage 2+3: v_d, v_t, ctx = v_t + attn0_bcast * v_d
                ctxT = fm.tile([128, KT, N], BF16, tag="ctxT")
                for mh in range(2):
                    wv_s = wv_sl[mh]
                    for mm in range(4):
                        m = 4 * mh + mm
                        pvd = ps_mm.tile([128, N], F32, tag="mm")
                        for k in range(KT):
                            nc.tensor.matmul(pvd,
                                             wv_s[:, k, 128 * mm:128 * (mm + 1)],
                                             dT[:, k, :],
                                             start=(k == 0), stop=(k == KT - 1))
                        pab = ps_ab.tile([128, N], F32)
                        nc.tensor.matmul(pab, e_s[:, m, :], attn0,
                                         start=True, stop=True)
                        # drain both PSUMs via ACT so the v-matmul pipeline
                        # never waits on the (LN-loaded) DVE queue
                        ab_s = work.tile([128, N], BF16, tag="ab_s")
                        nc.scalar.copy(ab_s, pab)
                        vd_s = work.tile([128, N], F32, tag="vd_s")
                        nc.scalar.copy(vd_s, pvd)
                        t1 = work.tile([128, N], F32, tag="t1")
                        nc.vector.tensor_mul(t1, ab_s, vd_s)
                        pvt = ps_mm.tile([128, N], F32, tag="mm")
                        for k in range(KT):
                            nc.tensor.matmul(pvt,
                                             wv_s[:, k, 128 * mm:128 * (mm + 1)],
                                             xtT[:, k, :],
                                             start=(k == 0), stop=(k == KT - 1))
                        nc.vector.tensor_add(ctxT[:, m, :], pvt, t1)

                # stage 4: y = w_o @ ctx + bod2 (feature-major)
                y_fm = []
                for mh in range(2):
                    wo_s = wo_sl[mh]
                    for mm in range(4):
                        m = 4 * mh + mm
                        pm = ps_mm.tile([128, N], F32, tag="mm")
                        for k in range(KT):
                            nc.tensor.matmul(pm,
                                             wo_s[:, k, 128 * mm:128 * (mm + 1)],
                                             ctxT[:, k, :],
                                             start=(k == 0), stop=(k == KT - 1))
                        ym = work.tile([128, N], F32, tag="y_fm", bufs=8)
                        nc.scalar.activation(ym, pm,
                                             mybir.ActivationFunctionType.Identity,
                                             bias=bod_s[:, m:m + 1])
                        y_fm.append(ym)

                # prefetch the first two FFN1 weight slices now -- the
                # sync DMA queue is otherwise idle until stage 6
                w1_pre = []
                for mg in range(2):
                    w1_s = wstream.tile([128, KT, 512], BF16, tag="wA",
                                        name="w1_s")
                    nc.sync.dma_start(
                        w1_s,
                        w1_d[:, mg // 2, :, 512 * (mg % 2):512 * (mg % 2 + 1)])
                    w1_pre.append(w1_s)

                # stage 5: row-major LN1 -> x_rm (f32) and xT (bf16 fm)
                # (emit all y transposes+LN chains first so PE stays busy
                # on later i transposes while earlier LN chains drain)
                x_rm = []
                xT = fm.tile([128, KT, N], BF16, tag="xT")
                for i in range(ni):
                    y_rm = rm.tile([128, D], F32, tag="y_rm")
                    for half in range(2):
                        ptr = ps_tr.tile([128, 512], F32, tag="tr")
                        for mm in range(4):
                            m = half * 4 + mm
                            nc.tensor.transpose(
                                ptr[:, 128 * mm:128 * (mm + 1)],
                                y_fm[m][:, 128 * i:128 * (i + 1)], ident)
                        nc.scalar.copy(
                            y_rm[:, 512 * half:512 * (half + 1)], ptr)
                    xi = rm.tile([128, D], F32, tag="x_rm", bufs=ni)
                    layernorm_rm(y_rm, xi,
                                 None if trivial_ln1 else g1_s,
                                 None if trivial_ln1 else c1_s)
                    x_rm.append(xi)
                for i in range(ni):
                    for j in range(KT):
                        ptr = ps_tr.tile([128, 128], F32, tag="tr")
                        nc.tensor.transpose(ptr,
                                            x_rm[i][:, 128 * j:128 * (j + 1)],
                                            ident)
                        nc.vector.tensor_copy(xT[:, j, 128 * i:128 * (i + 1)],
                                              ptr)

                # stage 6: FFN1, h = gelu(w1 @ x + b1) (feature-major bf16)
                hT = fm.tile([128, FMT, N], BF16, tag="hT")
                for mg in range(8):
                    if mg < 2:
                        w1_s = w1_pre[mg]
                    else:
                        w1_s = wstream.tile([128, KT, 512], BF16, tag="wA",
                                            name="w1_s")
                        nc.sync.dma_start(
                            w1_s,
                            w1_d[:, mg // 2, :,
                                 512 * (mg % 2):512 * (mg % 2 + 1)])
                    if blk + 1 < nb and mg >= 4:
                        nxt_vo.append(prefetch_vo(
                            ((0, wv_d), (1, wv_d), (0, wo_d), (1, wo_d))[mg - 4]))
                    for mm in range(4):
                        m = mg * 4 + mm
                        pm = ps_mm.tile([128, N], F32, tag="mm")
                        for k in range(KT):
                            nc.tensor.matmul(pm,
                                             w1_s[:, k, 128 * mm:128 * (mm + 1)],
                                             xT[:, k, :],
                                             start=(k == 0), stop=(k == KT - 1))
                        nc.scalar.activation(hT[:, m, :], pm, gelu_func,
                                             bias=b1_s[:, m:m + 1])

                # prefetch next block's inputs (DMA-transposes queue
                # behind the FFN1 weight stream, ahead of the FFN2 one)
                if blk + 1 < nb:
                    nxt = stage0(blk + 1)

                # stage 7: FFN2 + bias (feature-major)
                f_fm = []
                for m in range(MT):
                    pm = ps_mm.tile([128, N], F32, tag="mm")
                    for kh in range(2):
                        w2_s = wstream.tile([128, 16, 128], BF16, tag="w2s")
                        nc.sync.dma_start(w2_s,
                                          w2_d[:, m, 16 * kh:16 * (kh + 1), :])
                        for kk in range(16):
                            k = 16 * kh + kk
                            nc.tensor.matmul(pm, w2_s[:, kk, :], hT[:, k, :],
                                             start=(k == 0),
                                             stop=(k == FMT - 1))
                    fme = work.tile([128, N], F32, tag="y_fm", bufs=8)
                    nc.scalar.activation(fme, pm,
                                         mybir.ActivationFunctionType.Identity,
                                         bias=b2_s[:, m:m + 1])
                    f_fm.append(fme)

                # next block's scores ahead of LN2 so the sigmoid is
                # not queued behind the LN2 sqrt chain on ACT
                if blk + 1 < nb:
                    nxt_at = stage1(nxt[1])

                # stage 8: z = x + ffn (in place), LN2, store
                for i in range(ni):
                    f_rm = rm.tile([128, D], F32, tag="f_rm", bufs=2)
                    for half in range(2):
                        ptr = ps_tr.tile([128, 512], F32, tag="tr")
                        for mm in range(4):
                            m = half * 4 + mm
                            nc.tensor.transpose(
                                ptr[:, 128 * mm:128 * (mm + 1)],
                                f_fm[m][:, 128 * i:128 * (i + 1)], ident)
                        nc.scalar.copy(
                            f_rm[:, 512 * half:512 * (half + 1)], ptr)
                    nc.vector.tensor_add(x_rm[i], x_rm[i], f_rm)
                    o_i = rm.tile([128, D], F32, tag="o_rm")
                    layernorm_rm(x_rm[i], o_i,
                                 None if trivial_ln2 else g2_s,
                                 None if trivial_ln2 else c2_s)
                    nc.gpsimd.dma_start(
                        out_d[r0 + 128 * i:r0 + 128 * (i + 1), :], o_i)

    nc.compile()
    return nc


def host_prepare(inputs):
    """Fold parameters and lay out weights for the device (all O(params))."""
    f64 = {k: np.asarray(inputs[k], dtype=np.float64)
           for k in ("dom_movie", "w_q", "w_k", "b_q", "w_o", "b_o", "b_v")}
    qs = (f64["dom_movie"] @ f64["w_q"].T + f64["b_q"]) / np.sqrt(HD)  # (1, D)
    qh = qs.reshape(H, HD)
    A = np.einsum("hd,hdD->hD", qh, f64["w_k"].reshape(H, HD, D))  # (H, D)
    bod2 = f64["b_o"] + f64["dom_movie"][0] + f64["w_o"] @ f64["b_v"]  # (D,)

    E = np.zeros((H, MT, 128), np.float32)
    for m in range(MT):
        for p in range(128):
            E[2 * m + p // 64, m, p] = 1.0

    w_v = np.asarray(inputs["w_v"], np.float32)
    w_o = np.asarray(inputs["w_o"], np.float32)
    w1 = np.asarray(inputs["w1"], np.float32)
    w2 = np.asarray(inputs["w2"], np.float32)

    def fm_weight(wT):  # wT (d_in, d_out) -> [128, d_in/128, d_out]
        return np.ascontiguousarray(
            wT.reshape(-1, 128, wT.shape[1]).transpose(1, 0, 2)).astype(NPBF16)

    prep = {
        "wvT": fm_weight(w_v.T),
        "woT": fm_weight(w_o.T),
        "w1P": np.ascontiguousarray(
            w1.T.reshape(KT, 128, NMG, D).transpose(1, 2, 0, 3)).astype(NPBF16),
        "w2P": np.ascontiguousarray(
            w2.T.reshape(FMT, 128, MT, 128).transpose(1, 2, 0, 3)).astype(NPBF16),
        "AT": np.ascontiguousarray(
            A.T.reshape(KT, 128, H).transpose(1, 0, 2)).astype(NPBF16),
        "E": E.astype(NPBF16),
        "bod2": np.ascontiguousarray(
            bod2.reshape(MT, 128).T).astype(np.float32),
        "b1p": np.ascontiguousarray(
            np.asarray(inputs["b1"], np.float64).reshape(FMT, 128).T
        ).astype(np.float32),
        "b2p": np.ascontiguousarray(
            np.asarray(inputs["b2"], np.float64).reshape(MT, 128).T
        ).astype(np.float32),
    }
    trivial_ln1 = bool(np.all(np.asarray(inputs["ln1_g"]) == 1.0)
                       and np.all(np.asarray(inputs["ln1_b"]) == 0.0))
    trivial_ln2 = bool(np.all(np.asarray(inputs["ln2_g"]) == 1.0)
                       and np.all(np.asarray(inputs["ln2_b"]) == 0.0))
    if not trivial_ln1:
        prep["g1"] = np.asarray(inputs["ln1_g"], np.float32)
        prep["c1"] = np.asarray(inputs["ln1_b"], np.float32)
    if not trivial_ln2:
        prep["g2"] = np.asarray(inputs["ln2_g"], np.float32)
        prep["c2"] = np.asarray(inputs["ln2_b"], np.float32)
    return prep, trivial_ln1, trivial_ln2


_PROGRAM_CACHE = {}


def _get_program(b_core, r_blk, t1, t2):
    key = (b_core, r_blk, t1, t2)
    if key not in _PROGRAM_CACHE:
        _PROGRAM_CACHE[key] = build_program(b_core, r_blk, t1, t2)
    return _PROGRAM_CACHE[key]


def kernel(h_u_cross, h_u_target, dom_movie, w_q, w_k, w_v, b_q, b_k, b_v,
           w_o, b_o, ln1_g, ln1_b, w1, b1, w2, b2, ln2_g, ln2_b,
           trace=False, r_blk=512, **run_kwargs):
    inputs = dict(h_u_cross=h_u_cross, h_u_target=h_u_target,
                  dom_movie=dom_movie, w_q=w_q, w_k=w_k, w_v=w_v, b_q=b_q,
                  b_k=b_k, b_v=b_v, w_o=w_o, b_o=b_o, ln1_g=ln1_g,
                  ln1_b=ln1_b, w1=w1, b1=b1, w2=w2, b2=b2, ln2_g=ln2_g,
                  ln2_b=ln2_b)
    prep, t1, t2 = host_prepare(inputs)
    nc = _get_program(B_CORE, r_blk, t1, t2)

    xc = np.asarray(h_u_cross, np.float32)
    xt = np.asarray(h_u_target, np.float32)
    xtb = np.ascontiguousarray(xt.astype(NPBF16))
    db = np.ascontiguousarray((xc - xt).astype(NPBF16))
    in_maps = []
    for c in range(N_CORES):
        m = dict(prep)
        m["xtb"] = xtb[c * B_CORE:(c + 1) * B_CORE]
        m["db"] = db[c * B_CORE:(c + 1) * B_CORE]
        in_maps.append(m)

    res = run_bass_kernel_spmd(nc, in_maps, core_ids=list(range(N_CORES)),
                               trace=trace, **run_kwargs)
    out = np.concatenate([res.results[c]["out"] for c in range(N_CORES)], axis=0)
    kernel.last_results = res
    return out.astype(np.float32)
